# revision 7
# baseline (speedup 1.0000x reference)
"""Trainium2 Bass kernel for nn_AlarmworkRNN: 2-track tanh RNN (v2).

Math (per reference):
  for l in 0..L-1:
      z1n = tanh(X[:,l] @ W_in1.T + b1 + (z1 + z2) @ W_rec1.T)
      z2n = tanh(X[:,l] @ W_in2.T + b2 + z2 @ W_rec2.T)  if l even else z2
      z1, z2 = z1n, z2n
  out = tanh(z1 @ W_out.T + b_out)       (computed on host, O=1)

Strategy (v2, fp16, "output-split" column tiling):
  Data-parallel over batch (8 cores x 64 rows).  State is held transposed
  (z12T, z2T: [H=1024 -> 8 k-tiles of 128, B=64]) as the matmul stationary;
  host-pretransposed weights are the moving operand, resident in SBUF.  The
  input projection X[l] @ W_in.T joins the same PSUM accumulation as 2 extra
  k-tiles (stationary = host-pretransposed X[l].T).

  Unlike v1 (which split the k-entries across the two PE column groups so the
  chains had to run serially per step and the two PSUM halves then needed an
  ACT copy + DVE add), v2 splits the OUTPUT columns: PE column strip g
  computes output columns [g*512,(g+1)*512) for ALL k-tiles.  Each strip's
  accumulation chain lives in its own PSUM bank (one pending group per bank
  is a hard HW/sim rule), and the two chains are emitted interleaved so they
  stream CONCURRENTLY through the two halves of the PE array -- per step the
  PE streams 10 x 512 columns instead of 20 x 512.

  The step's pre-activation lands "diagonally": bank0 partitions 0-63 hold
  cols 0-511, bank1 partitions 64-127 hold cols 512-1023.  ACT tanh maps
  both into one "stacked" SBUF tile [128, 512] (chunked so transposes/adds
  pipeline); PE transposes turn the stacked halves into z1nT k-tiles and a
  DVE add (z1nT + z2T) forms the next stationary.  The z-entry order
  (0,1,4,5,2,3,6,7) matches tanh-chunk completion so the next step's matmuls
  become ready progressively.

  The z2 track updates only on even steps; its matmul group is emitted
  split around the z1 chain as PE fill for the tanh/transpose/add latency.
"""
import numpy as np

B, L, I, H = 512, 512, 256, 1024
NC = 8
BC = B // NC          # 64 batch rows per core
KH = H // 128         # 8 hidden k-tiles
KI = I // 128         # 2 input k-tiles

N_Z2A = 5             # z2-group entries emitted at the opening (even) step
ZORD = (0, 1, 4, 5, 2, 3, 6, 7)   # z-entry order (tanh-chunk completion order)
KORD = (0, 1, 4, 5, 2, 3, 6, 7)   # transpose/add k-tile order

_CACHE = {}


def _build(L_steps, with_bias, reps=1):
    import concourse.bacc as bacc
    import concourse.tile as tile
    import concourse.mybir as mybir

    F32 = mybir.dt.float32
    DT = mybir.dt.float16
    Tanh = mybir.ActivationFunctionType.Tanh

    nc = bacc.Bacc("TRN2", target_bir_lowering=False)
    XT = nc.declare_dram_parameter("XT", [L_steps, 128, KI * BC], DT, isOutput=False)
    W1T = nc.declare_dram_parameter("W1T", [H, H], DT, isOutput=False)
    W2T = nc.declare_dram_parameter("W2T", [H, H], DT, isOutput=False)
    Wi1T = nc.declare_dram_parameter("Wi1T", [I, H], DT, isOutput=False)
    Wi2T = nc.declare_dram_parameter("Wi2T", [I, H], DT, isOutput=False)
    IDN = nc.declare_dram_parameter("IDN", [64, 64], DT, isOutput=False)
    if with_bias:
        BIA = nc.declare_dram_parameter("BIA", [2, H], DT, isOutput=False)
        ONE = nc.declare_dram_parameter("ONE", [1, BC], DT, isOutput=False)
    OUT = nc.declare_dram_parameter("OUT", [BC, H], F32, isOutput=True)

    with tile.TileContext(nc) as tc:
        with tc.tile_pool(name="const", bufs=1) as cpool, \
             tc.tile_pool(name="xt", bufs=6) as xpool, \
             tc.tile_pool(name="st", bufs=3) as spool, \
             tc.tile_pool(name="actt", bufs=3) as apool, \
             tc.tile_pool(name="fin", bufs=1) as fpool, \
             tc.tile_pool(name="ps1", bufs=2, space="PSUM") as ps1pool, \
             tc.tile_pool(name="ps2", bufs=1, space="PSUM") as ps2pool, \
             tc.tile_pool(name="pst", bufs=2, space="PSUM") as pstpool:

            # ---- resident weights: [128, ktile*H] with ktile-major free layout
            w1t_sb = cpool.tile([128, KH * H], DT)
            w2t_sb = cpool.tile([128, KH * H], DT)
            wi1t_sb = cpool.tile([128, KI * H], DT)
            wi2t_sb = cpool.tile([128, KI * H], DT)
            id_sb = cpool.tile([64, 64], DT)
            nc.sync.dma_start(id_sb[:], IDN[:])
            for k in range(KH):
                nc.sync.dma_start(w1t_sb[:, k*H:(k+1)*H], W1T[k*128:(k+1)*128, :])
                nc.sync.dma_start(w2t_sb[:, k*H:(k+1)*H], W2T[k*128:(k+1)*128, :])
            for k in range(KI):
                nc.sync.dma_start(wi1t_sb[:, k*H:(k+1)*H], Wi1T[k*128:(k+1)*128, :])
                nc.sync.dma_start(wi2t_sb[:, k*H:(k+1)*H], Wi2T[k*128:(k+1)*128, :])
            if with_bias:
                bia1_sb = cpool.tile([1, H], DT)
                bia2_sb = cpool.tile([1, H], DT)
                one_sb = cpool.tile([1, BC], DT)
                nc.sync.dma_start(bia1_sb[:], BIA[0:1, :])
                nc.sync.dma_start(bia2_sb[:], BIA[1:2, :])
                nc.sync.dma_start(one_sb[:], ONE[:])

            # ---- XT prefetch
            xts = {}

            def fetch_xt(l):
                if l >= L_steps:
                    return
                t = xpool.tile([128, KI * BC], DT, tag="xt")
                nc.sync.dma_start(t[:], XT[l])
                xts[l] = t

            class Group:
                """One step's PSUM accumulation.  PE column strip g streams
                output columns [g*512,(g+1)*512) of every entry into its own
                PSUM bank: ps[g*64:(g+1)*64, g*512:(g+1)*512].  The two strip
                chains are emitted interleaved so they run concurrently.

                Entry order: [bias?] + KI x-tiles + ZORD z-tiles.  bias/X are
                state-independent and can be emitted early; zT is set before
                the z entries are emitted."""

                def __init__(self, ps, xt_t, wi_sb, w_sb, bias_sb, nz=KH):
                    self.ps, self.xt, self.wi, self.w = ps, xt_t, wi_sb, w_sb
                    self.bias = bias_sb
                    self.zT = None
                    self.nz = nz
                    self.done = 0

                @property
                def nb(self):
                    return 1 if self.bias is not None else 0

                @property
                def n_open(self):
                    return self.nb + KI   # state-independent prefix

                def entry(self, i):
                    if i < self.nb:
                        return one_sb[0:1, :], self.bias, 0
                    i -= self.nb
                    if i < KI:
                        return self.xt[:, i*BC:(i+1)*BC], self.wi, i
                    k = ZORD[i - KI]
                    return self.zT[:, k*BC:(k+1)*BC], self.w, k

                def emit(self, hi=None):
                    n = self.nb + KI + self.nz
                    hi = n if hi is None else min(hi, n)
                    for i in range(self.done, hi):
                        stat, mov, k = self.entry(i)
                        for g in range(2):
                            nc.tensor.matmul(
                                self.ps[g*BC:(g+1)*BC, g*512:(g+1)*512],
                                stat, mov[:, k*H + g*512: k*H + g*512 + 512],
                                start=(i == 0), stop=(i == n - 1),
                                tile_position=(0, g*BC))
                    self.done = max(self.done, hi)

            def open_z1(l, nz=KH):
                ps = ps1pool.tile([128, H], F32, tag="ps1")
                return Group(ps, xts[l], wi1t_sb, w1t_sb,
                             bia1_sb if with_bias else None, nz)

            def open_z2(l, zT, nz=KH):
                ps = ps2pool.tile([128, H], F32, tag="ps2")
                g = Group(ps, xts[l], wi2t_sb, w2t_sb,
                          bia2_sb if with_bias else None, nz)
                g.zT = zT
                return g

            def tanh_step(ps, halves, nchunk=2):
                """halves[h][:, c] = tanh(ps[h*64:(h+1)*64, h*512 + c]):
                half h holds z[:, h*512:(h+1)*512] on partitions 0-63.
                Chunk order (h0,c0),(h1,c0),(h0,c1),(h1,c1) so the transposes
                for KORD k-tiles become ready progressively."""
                cw = 512 // nchunk
                for c in range(nchunk):
                    for h in range(2):
                        nc.scalar.activation(
                            halves[h][:, c*cw:(c+1)*cw],
                            ps[h*BC:(h+1)*BC, h*512 + c*cw: h*512 + (c+1)*cw],
                            Tanh)

            def transposes(halves, pst):
                # halves[h][b, c] = z[b, h*512 + c]
                for kk in KORD:
                    h, j = kk // 4, kk % 4
                    nc.tensor.transpose(pst[:, kk*BC:(kk+1)*BC],
                                        halves[h][:, j*128:(j+1)*128],
                                        id_sb[:])

            def z2_post(g2):
                """tanh + transposes + copy -> new pending z2T tile."""
                z2n = [apool.tile([BC, 512], DT, tag="z2na", name="z2na"),
                       apool.tile([BC, 512], DT, tag="z2nb", name="z2nb")]
                tanh_step(g2.ps, z2n, nchunk=1)
                pst2 = pstpool.tile([128, KH * BC], DT, tag="pst")
                transposes(z2n, pst2)
                z2T_new = spool.tile([128, KH * BC], DT, tag="z2T")
                nc.vector.tensor_copy(z2T_new[:], pst2[:])
                return z2T_new

            def add_z12(pst1, z2T):
                z12T = spool.tile([128, KH * BC], DT, tag="z12T")
                for p in range(0, KH, 2):
                    a = KORD[p]
                    nc.vector.tensor_add(z12T[:, a*BC:(a+2)*BC],
                                         pst1[:, a*BC:(a+2)*BC],
                                         z2T[:, a*BC:(a+2)*BC])
                return z12T

            def body():
                nonlocal xts
                xts = {}
                for l in range(min(3, L_steps)):
                    fetch_xt(l)
                # step 0: no recurrent state -> X(+bias)-only groups
                g1 = open_z1(0, nz=0)
                g1.emit()
                g2 = open_z2(0, None, nz=0)
                g2.emit()
                z2T_pending = z2_post(g2)
                g2 = None
                z2T = None

                for l in range(L_steps):
                    even = (l % 2 == 0)
                    last = (l == L_steps - 1)
                    fetch_xt(l + 3)

                    # close this step's z1 accumulation
                    g1.emit()

                    # z2 state after step l (updated on even steps)
                    if even:
                        z2T = z2T_pending

                    # finish the z2 matmul group for step l+1 (PE fill)
                    if (not last) and (l % 2 == 1) and g2 is not None:
                        g2.emit()

                    # tanh of this step's z1
                    if last:
                        fin = [fpool.tile([BC, 512], F32, tag="fina", name="fina"),
                               fpool.tile([BC, 512], F32, tag="finb", name="finb")]
                        tanh_step(g1.ps, fin, nchunk=1)
                        nc.sync.dma_start(OUT[:, 0:512], fin[0][:])
                        nc.sync.dma_start(OUT[:, 512:1024], fin[1][:])
                        break
                    z1n = [apool.tile([BC, 512], DT, tag="z1na", name="z1na"),
                           apool.tile([BC, 512], DT, tag="z1nb", name="z1nb")]
                    tanh_step(g1.ps, z1n)

                    # open next step's z1 group; emit state-independent prefix
                    g1n = open_z1(l + 1)
                    g1n.emit(g1n.n_open)

                    # open the z2 group for step l+2 at the even-step tail --
                    # ahead of the tanh-gated transposes in the PE FIFO, so its
                    # ready entries fill the tanh/transpose/add wait
                    if even and l + 2 < L_steps:
                        g2 = open_z2(l + 2, z2T)
                        g2.emit(g2.nb + N_Z2A)

                    # transpose z1n -> z1nT k-tiles (PSUM)
                    pst1 = pstpool.tile([128, KH * BC], DT, tag="pst")
                    transposes(z1n, pst1)

                    # z12T = z1nT + z2T(after this step) -- emitted before
                    # z2_post so the critical adds lead the DVE FIFO
                    z12T = add_z12(pst1, z2T)
                    g1n.zT = z12T
                    g1 = g1n

                    # z2 epilogue for step l+1 (tanh queues behind tanh_z1 on
                    # ACT; its transposes fill the PE after z1's)
                    if (not last) and (l % 2 == 1) and g2 is not None:
                        z2T_pending = z2_post(g2)
                        g2 = None

                    if l >= 1:
                        xts.pop(l - 1, None)

            if reps > 1:
                with tc.For_i(0, reps, 1):
                    body()
            else:
                body()
    nc.compile()
    return nc


def _get_nc(L_steps, with_bias, reps=1):
    key = (L_steps, with_bias, reps)
    if key not in _CACHE:
        _CACHE[key] = _build(L_steps, with_bias, reps)
    return _CACHE[key]


def _prep_in_maps(X, W_in1, b_in1, W_rec1, W_in2, b_in2, W_rec2, L_steps):
    dt = np.float16
    with_bias = bool(np.any(b_in1) or np.any(b_in2))
    w1t = np.ascontiguousarray(W_rec1.T.astype(dt))
    w2t = np.ascontiguousarray(W_rec2.T.astype(dt))
    wi1t = np.ascontiguousarray(W_in1.T.astype(dt))
    wi2t = np.ascontiguousarray(W_in2.T.astype(dt))
    idn = np.eye(64, dtype=dt)
    in_maps = []
    for c in range(NC):
        xt = np.ascontiguousarray(
            X[c*BC:(c+1)*BC, :L_steps, :].transpose(1, 2, 0)
            .reshape(L_steps, KI, 128, BC).transpose(0, 2, 1, 3)
            .reshape(L_steps, 128, KI * BC).astype(dt))
        m = {"XT": xt, "W1T": w1t, "W2T": w2t, "Wi1T": wi1t, "Wi2T": wi2t,
             "IDN": idn}
        if with_bias:
            m["BIA"] = np.ascontiguousarray(
                np.stack([b_in1[:, 0], b_in2[:, 0]]).astype(dt))
            m["ONE"] = np.ones((1, BC), dt)
        in_maps.append(m)
    return in_maps, with_bias


def run_device(X, W_in1, b_in1, W_rec1, W_in2, b_in2, W_rec2, L_steps=L):
    """Run the recurrence on 8 cores; returns z1_final (B, H) float32."""
    from concourse.bass_utils import run_bass_kernel_spmd
    in_maps, with_bias = _prep_in_maps(X, W_in1, b_in1, W_rec1, W_in2, b_in2,
                                       W_rec2, L_steps)
    nc = _get_nc(L_steps, with_bias, 1)
    res = run_bass_kernel_spmd(nc, in_maps, list(range(NC)))
    return np.concatenate([res.results[c]["OUT"] for c in range(NC)], axis=0)


def kernel(X, W_in1, b_in1, W_rec1, W_in2, b_in2, W_rec2, W_out, b_out):
    X = np.asarray(X); W_out = np.asarray(W_out); b_out = np.asarray(b_out)
    assert X.shape == (B, L, I), f"unexpected X shape {X.shape}"
    z1 = run_device(X, np.asarray(W_in1), np.asarray(b_in1),
                    np.asarray(W_rec1), np.asarray(W_in2), np.asarray(b_in2),
                    np.asarray(W_rec2))
    out = np.tanh(z1.astype(np.float64) @ W_out.astype(np.float64).T
                  + b_out.astype(np.float64)[:, 0])
    return out.reshape(B, 1).astype(np.float32)


# revision 9
# speedup vs baseline: 1.0105x; 1.0105x over previous
"""Trainium2 Bass kernel for nn_AlarmworkRNN: 2-track tanh RNN (v6).

Math (per reference):
  for l in 0..L-1:
      z1n = tanh(X[:,l] @ W_in1.T + b1 + (z1 + z2) @ W_rec1.T)
      z2n = tanh(X[:,l] @ W_in2.T + b2 + z2 @ W_rec2.T)  if l even else z2
      z1, z2 = z1n, z2n
  out = tanh(z1 @ W_out.T + b_out)       (computed on host, O=1)

Strategy (fp16, "output-split" column tiling, stacked activations):
  Data-parallel over batch (8 cores x 64 rows).  State is held transposed
  (z12T, z2T: [128, 8 k-tiles x 64]) as the matmul stationary; host-
  pretransposed weights are the moving operand, resident in SBUF.  The input
  projection X[l] @ W_in.T joins the same PSUM accumulation as 2 extra
  k-tiles (stationary = host-pretransposed X[l].T).

  PE column strip g computes output columns [g*512,(g+1)*512) for ALL
  k-tiles; each strip's accumulation chain lives in its own PSUM bank (one
  pending group per bank is a hard HW rule) and the two chains are emitted
  interleaved so they stream CONCURRENTLY through the two halves of the PE
  array: per step the PE streams 10 x 512 columns instead of 20 x 512.

  The pre-activation lands "diagonally" (bank0 parts 0-63 = cols 0-511,
  bank1 parts 64-127 = cols 512-1023).  ACT tanh maps both halves into ONE
  stacked SBUF tile [128, 512] (partitions 64-127 hold cols 512-1023), so
  transposes run as 4 full-row [128,128] PE ops per step -- each writes a
  PAIR of z1nT k-tiles (j, j+4) -- and the z1nT + z2T add is 2 DVE ops.
  State tiles use the pair-interleaved layout SORD = (0,4,1,5,2,6,3,7);
  matmul z-entries are emitted in that order to match tanh-chunk completion.

  The z2 track updates only on even steps; its matmul group is emitted
  split around the z1 chain as PE fill for the tanh/transpose/add latency.
"""
import numpy as np

B, L, I, H = 512, 512, 256, 1024
NC = 8
BC = B // NC          # 64 batch rows per core
KH = H // 128         # 8 hidden k-tiles
KI = I // 128         # 2 input k-tiles

N_Z2A = 5             # z2-group entries emitted at the opening (even) step
SORD = (0, 4, 1, 5, 2, 6, 3, 7)           # k-tile order in pst/z12T/z2T layout
POS = {k: i for i, k in enumerate(SORD)}  # k-tile -> 64-col slot index

_CACHE = {}


def _build(L_steps, with_bias, reps=1):
    import concourse.bacc as bacc
    import concourse.tile as tile
    import concourse.mybir as mybir

    F32 = mybir.dt.float32
    DT = mybir.dt.float16
    Tanh = mybir.ActivationFunctionType.Tanh

    nc = bacc.Bacc("TRN2", target_bir_lowering=False)
    XT = nc.declare_dram_parameter("XT", [L_steps, 128, KI * BC], DT, isOutput=False)
    W1T = nc.declare_dram_parameter("W1T", [H, H], DT, isOutput=False)
    W2T = nc.declare_dram_parameter("W2T", [H, H], DT, isOutput=False)
    Wi1T = nc.declare_dram_parameter("Wi1T", [I, H], DT, isOutput=False)
    Wi2T = nc.declare_dram_parameter("Wi2T", [I, H], DT, isOutput=False)
    IDN = nc.declare_dram_parameter("IDN", [128, 128], DT, isOutput=False)
    if with_bias:
        BIA = nc.declare_dram_parameter("BIA", [2, H], DT, isOutput=False)
        ONE = nc.declare_dram_parameter("ONE", [1, BC], DT, isOutput=False)
    OUT = nc.declare_dram_parameter("OUT", [BC, H], F32, isOutput=True)

    with tile.TileContext(nc) as tc:
        with tc.tile_pool(name="const", bufs=1) as cpool, \
             tc.tile_pool(name="xt", bufs=6) as xpool, \
             tc.tile_pool(name="st", bufs=3) as spool, \
             tc.tile_pool(name="actt", bufs=3) as apool, \
             tc.tile_pool(name="fin", bufs=1) as fpool, \
             tc.tile_pool(name="ps1", bufs=2, space="PSUM") as ps1pool, \
             tc.tile_pool(name="ps2", bufs=1, space="PSUM") as ps2pool, \
             tc.tile_pool(name="pst", bufs=2, space="PSUM") as pstpool:

            # ---- resident weights: [128, ktile*H] with ktile-major free layout
            w1t_sb = cpool.tile([128, KH * H], DT)
            w2t_sb = cpool.tile([128, KH * H], DT)
            wi1t_sb = cpool.tile([128, KI * H], DT)
            wi2t_sb = cpool.tile([128, KI * H], DT)
            id_sb = cpool.tile([128, 128], DT)
            nc.sync.dma_start(id_sb[:], IDN[:])
            for k in range(KH):
                nc.sync.dma_start(w1t_sb[:, k*H:(k+1)*H], W1T[k*128:(k+1)*128, :])
                nc.sync.dma_start(w2t_sb[:, k*H:(k+1)*H], W2T[k*128:(k+1)*128, :])
            for k in range(KI):
                nc.sync.dma_start(wi1t_sb[:, k*H:(k+1)*H], Wi1T[k*128:(k+1)*128, :])
                nc.sync.dma_start(wi2t_sb[:, k*H:(k+1)*H], Wi2T[k*128:(k+1)*128, :])
            if with_bias:
                bia1_sb = cpool.tile([1, H], DT)
                bia2_sb = cpool.tile([1, H], DT)
                one_sb = cpool.tile([1, BC], DT)
                nc.sync.dma_start(bia1_sb[:], BIA[0:1, :])
                nc.sync.dma_start(bia2_sb[:], BIA[1:2, :])
                nc.sync.dma_start(one_sb[:], ONE[:])

            # ---- XT prefetch (one DMA per step)
            xts = {}

            def fetch_xt(l):
                if l >= L_steps:
                    return
                t = xpool.tile([128, KI * BC], DT, tag="xt")
                nc.sync.dma_start(t[:], XT[l])
                xts[l] = t

            class Group:
                """One step's PSUM accumulation.  PE column strip g streams
                output columns [g*512,(g+1)*512) of every entry into its own
                PSUM bank: ps[g*64:(g+1)*64, g*512:(g+1)*512].  The two strip
                chains are emitted interleaved so they run concurrently.

                Entry order: [bias?] + KI x-tiles + SORD z-tiles.  bias/X are
                state-independent and can be emitted early; zT is set before
                the z entries are emitted."""

                def __init__(self, ps, xt_t, wi_sb, w_sb, bias_sb, nz=KH):
                    self.ps, self.xt, self.wi, self.w = ps, xt_t, wi_sb, w_sb
                    self.bias = bias_sb
                    self.zT = None
                    self.nz = nz
                    self.done = 0

                @property
                def nb(self):
                    return 1 if self.bias is not None else 0

                @property
                def n_open(self):
                    return self.nb + KI   # state-independent prefix

                def entry(self, i):
                    if i < self.nb:
                        return one_sb[0:1, :], self.bias, 0
                    i -= self.nb
                    if i < KI:
                        return self.xt[:, i*BC:(i+1)*BC], self.wi, i
                    k = SORD[i - KI]
                    return self.zT[:, POS[k]*BC:(POS[k]+1)*BC], self.w, k

                def emit(self, hi=None):
                    n = self.nb + KI + self.nz
                    hi = n if hi is None else min(hi, n)
                    for i in range(self.done, hi):
                        stat, mov, k = self.entry(i)
                        for g in range(2):
                            nc.tensor.matmul(
                                self.ps[g*BC:(g+1)*BC, g*512:(g+1)*512],
                                stat, mov[:, k*H + g*512: k*H + g*512 + 512],
                                start=(i == 0), stop=(i == n - 1),
                                tile_position=(0, g*BC))
                    self.done = max(self.done, hi)

            def open_z1(l, nz=KH):
                ps = ps1pool.tile([128, H], F32, tag="ps1")
                return Group(ps, xts[l], wi1t_sb, w1t_sb,
                             bia1_sb if with_bias else None, nz)

            def open_z2(l, zT, nz=KH):
                ps = ps2pool.tile([128, H], F32, tag="ps2")
                g = Group(ps, xts[l], wi2t_sb, w2t_sb,
                          bia2_sb if with_bias else None, nz)
                g.zT = zT
                return g

            def tanh_step(ps, dst, nchunk=2):
                """dst[h*64:(h+1)*64, c] = tanh(ps[h*64:(h+1)*64, h*512 + c]):
                one stacked [128, 512] tile -- partitions 64-127 hold columns
                512-1023.  Chunk order (h0,c0),(h1,c0),(h0,c1),(h1,c1) so the
                paired transposes become ready progressively."""
                cw = 512 // nchunk
                for c in range(nchunk):
                    for h in range(2):
                        nc.scalar.activation(
                            dst[h*BC:(h+1)*BC, c*cw:(c+1)*cw],
                            ps[h*BC:(h+1)*BC, h*512 + c*cw: h*512 + (c+1)*cw],
                            Tanh)

            def transposes(stacked, pst):
                """4 full-row transposes; chunk j writes the k-tile PAIR
                (j, j+4) at pst cols [j*128,(j+1)*128) (SORD layout)."""
                for j in range(4):
                    nc.tensor.transpose(pst[:, j*128:(j+1)*128],
                                        stacked[:, j*128:(j+1)*128],
                                        id_sb[:])

            def z2_post(g2):
                """tanh + transposes + copy -> new pending z2T tile."""
                z2n = apool.tile([128, 512], DT, tag="z2n")
                tanh_step(g2.ps, z2n, nchunk=1)
                pst2 = pstpool.tile([128, KH * BC], DT, tag="pst")
                transposes(z2n, pst2)
                z2T_new = spool.tile([128, KH * BC], DT, tag="z2T")
                nc.vector.tensor_copy(z2T_new[:], pst2[:])
                return z2T_new

            def add_z12(pst1, z2T):
                z12T = spool.tile([128, KH * BC], DT, tag="z12T")
                for c in range(2):
                    nc.vector.tensor_add(z12T[:, c*256:(c+1)*256],
                                         pst1[:, c*256:(c+1)*256],
                                         z2T[:, c*256:(c+1)*256])
                return z12T

            def body():
                nonlocal xts
                xts = {}
                for l in range(min(3, L_steps)):
                    fetch_xt(l)
                # step 0: no recurrent state -> X(+bias)-only groups
                g1 = open_z1(0, nz=0)
                g1.emit()
                g2 = open_z2(0, None, nz=0)
                g2.emit()
                z2T_pending = z2_post(g2)
                g2 = None
                z2T = None

                for l in range(L_steps):
                    even = (l % 2 == 0)
                    last = (l == L_steps - 1)
                    fetch_xt(l + 3)

                    # close this step's z1 accumulation
                    g1.emit()

                    # z2 state after step l (updated on even steps)
                    if even:
                        z2T = z2T_pending

                    # finish the z2 matmul group for step l+1 (PE fill)
                    if (not last) and (l % 2 == 1) and g2 is not None:
                        g2.emit()

                    # tanh of this step's z1
                    if last:
                        fin = fpool.tile([128, 512], F32)
                        tanh_step(g1.ps, fin, nchunk=1)
                        nc.sync.dma_start(OUT[:, 0:512], fin[0:BC, :])
                        nc.sync.dma_start(OUT[:, 512:1024], fin[BC:128, :])
                        break
                    z1n = apool.tile([128, 512], DT, tag="z1n")
                    tanh_step(g1.ps, z1n)

                    # open next step's z1 group; emit state-independent prefix
                    g1n = open_z1(l + 1)
                    g1n.emit(g1n.n_open)

                    # open the z2 group for step l+2 at the even-step tail --
                    # ahead of the tanh-gated transposes in the PE FIFO, so its
                    # ready entries fill the tanh/transpose/add wait
                    if even and l + 2 < L_steps:
                        g2 = open_z2(l + 2, z2T)
                        g2.emit(g2.nb + N_Z2A)

                    # transpose z1n -> z1nT k-tile pairs (PSUM, SORD layout)
                    pst1 = pstpool.tile([128, KH * BC], DT, tag="pst")
                    transposes(z1n, pst1)

                    # z2 epilogue for step l+1 (tanh_z2 queues behind tanh_z1 on
                    # ACT; its transposes fill the PE while DVE adds below)
                    if (not last) and (l % 2 == 1) and g2 is not None:
                        z2T_pending = z2_post(g2)
                        g2 = None

                    # z12T = z1nT + z2T(after this step)
                    z12T = add_z12(pst1, z2T)
                    g1n.zT = z12T
                    g1 = g1n

                    if l >= 1:
                        xts.pop(l - 1, None)

            if reps > 1:
                with tc.For_i(0, reps, 1):
                    body()
            else:
                body()
    nc.compile()
    return nc


def _get_nc(L_steps, with_bias, reps=1):
    key = (L_steps, with_bias, reps)
    if key not in _CACHE:
        _CACHE[key] = _build(L_steps, with_bias, reps)
    return _CACHE[key]


def _prep_in_maps(X, W_in1, b_in1, W_rec1, W_in2, b_in2, W_rec2, L_steps):
    dt = np.float16
    with_bias = bool(np.any(b_in1) or np.any(b_in2))
    w1t = np.ascontiguousarray(W_rec1.T.astype(dt))
    w2t = np.ascontiguousarray(W_rec2.T.astype(dt))
    wi1t = np.ascontiguousarray(W_in1.T.astype(dt))
    wi2t = np.ascontiguousarray(W_in2.T.astype(dt))
    idn = np.eye(128, dtype=dt)
    in_maps = []
    for c in range(NC):
        xt = np.ascontiguousarray(
            X[c*BC:(c+1)*BC, :L_steps, :].transpose(1, 2, 0)
            .reshape(L_steps, KI, 128, BC).transpose(0, 2, 1, 3)
            .reshape(L_steps, 128, KI * BC).astype(dt))
        m = {"XT": xt, "W1T": w1t, "W2T": w2t, "Wi1T": wi1t, "Wi2T": wi2t,
             "IDN": idn}
        if with_bias:
            m["BIA"] = np.ascontiguousarray(
                np.stack([b_in1[:, 0], b_in2[:, 0]]).astype(dt))
            m["ONE"] = np.ones((1, BC), dt)
        in_maps.append(m)
    return in_maps, with_bias


def run_device(X, W_in1, b_in1, W_rec1, W_in2, b_in2, W_rec2, L_steps=L):
    """Run the recurrence on 8 cores; returns z1_final (B, H) float32."""
    from concourse.bass_utils import run_bass_kernel_spmd
    in_maps, with_bias = _prep_in_maps(X, W_in1, b_in1, W_rec1, W_in2, b_in2,
                                       W_rec2, L_steps)
    nc = _get_nc(L_steps, with_bias, 1)
    res = run_bass_kernel_spmd(nc, in_maps, list(range(NC)))
    return np.concatenate([res.results[c]["OUT"] for c in range(NC)], axis=0)


def kernel(X, W_in1, b_in1, W_rec1, W_in2, b_in2, W_rec2, W_out, b_out):
    X = np.asarray(X); W_out = np.asarray(W_out); b_out = np.asarray(b_out)
    assert X.shape == (B, L, I), f"unexpected X shape {X.shape}"
    z1 = run_device(X, np.asarray(W_in1), np.asarray(b_in1),
                    np.asarray(W_rec1), np.asarray(W_in2), np.asarray(b_in2),
                    np.asarray(W_rec2))
    out = np.tanh(z1.astype(np.float64) @ W_out.astype(np.float64).T
                  + b_out.astype(np.float64)[:, 0])
    return out.reshape(B, 1).astype(np.float32)


# revision 10
# speedup vs baseline: 1.0247x; 1.0141x over previous
"""Trainium2 Bass kernel for nn_AlarmworkRNN: 2-track tanh RNN (v2).

Math (per reference):
  for l in 0..L-1:
      z1n = tanh(X[:,l] @ W_in1.T + b1 + (z1 + z2) @ W_rec1.T)
      z2n = tanh(X[:,l] @ W_in2.T + b2 + z2 @ W_rec2.T)  if l even else z2
      z1, z2 = z1n, z2n
  out = tanh(z1 @ W_out.T + b_out)       (computed on host, O=1)

Strategy (v2, fp16, "output-split" column tiling):
  Data-parallel over batch (8 cores x 64 rows).  State is held transposed
  (z12T, z2T: [H=1024 -> 8 k-tiles of 128, B=64]) as the matmul stationary;
  host-pretransposed weights are the moving operand, resident in SBUF.  The
  input projection X[l] @ W_in.T joins the same PSUM accumulation as 2 extra
  k-tiles (stationary = host-pretransposed X[l].T).

  Unlike v1 (which split the k-entries across the two PE column groups so the
  chains had to run serially per step and the two PSUM halves then needed an
  ACT copy + DVE add), v2 splits the OUTPUT columns: PE column strip g
  computes output columns [g*512,(g+1)*512) for ALL k-tiles.  Each strip's
  accumulation chain lives in its own PSUM bank (one pending group per bank
  is a hard HW/sim rule), and the two chains are emitted interleaved so they
  stream CONCURRENTLY through the two halves of the PE array -- per step the
  PE streams 10 x 512 columns instead of 20 x 512.

  The step's pre-activation lands "diagonally": bank0 partitions 0-63 hold
  cols 0-511, bank1 partitions 64-127 hold cols 512-1023.  ACT tanh maps
  both into one "stacked" SBUF tile [128, 512] (chunked so transposes/adds
  pipeline); PE transposes turn the stacked halves into z1nT k-tiles and a
  DVE add (z1nT + z2T) forms the next stationary.  The z-entry order
  (0,1,4,5,2,3,6,7) matches tanh-chunk completion so the next step's matmuls
  become ready progressively.

  The z2 track updates only on even steps; its matmul group is emitted
  split around the z1 chain as PE fill for the tanh/transpose/add latency.
"""
import numpy as np

B, L, I, H = 512, 512, 256, 1024
NC = 8
BC = B // NC          # 64 batch rows per core
KH = H // 128         # 8 hidden k-tiles
KI = I // 128         # 2 input k-tiles

N_Z2A = 5             # z2-group entries emitted at the opening (even) step
ZORD = (0, 1, 4, 5, 2, 3, 6, 7)   # z-entry order (tanh-chunk completion order)
KORD = (0, 1, 4, 5, 2, 3, 6, 7)   # transpose/add k-tile order

_CACHE = {}


def _build(L_steps, with_bias, reps=1):
    import concourse.bacc as bacc
    import concourse.tile as tile
    import concourse.mybir as mybir

    F32 = mybir.dt.float32
    DT = mybir.dt.float16
    Tanh = mybir.ActivationFunctionType.Tanh

    nc = bacc.Bacc("TRN2", target_bir_lowering=False)
    XT = nc.declare_dram_parameter("XT", [L_steps, 128, KI * BC], DT, isOutput=False)
    W1T = nc.declare_dram_parameter("W1T", [H, H], DT, isOutput=False)
    W2T = nc.declare_dram_parameter("W2T", [H, H], DT, isOutput=False)
    Wi1T = nc.declare_dram_parameter("Wi1T", [I, H], DT, isOutput=False)
    Wi2T = nc.declare_dram_parameter("Wi2T", [I, H], DT, isOutput=False)
    IDN = nc.declare_dram_parameter("IDN", [64, 64], DT, isOutput=False)
    if with_bias:
        BIA = nc.declare_dram_parameter("BIA", [2, H], DT, isOutput=False)
        ONE = nc.declare_dram_parameter("ONE", [1, BC], DT, isOutput=False)
    OUT = nc.declare_dram_parameter("OUT", [BC, H], F32, isOutput=True)

    with tile.TileContext(nc) as tc:
        with tc.tile_pool(name="const", bufs=1) as cpool, \
             tc.tile_pool(name="xt", bufs=6) as xpool, \
             tc.tile_pool(name="st", bufs=3) as spool, \
             tc.tile_pool(name="actt", bufs=3) as apool, \
             tc.tile_pool(name="fin", bufs=1) as fpool, \
             tc.tile_pool(name="ps1", bufs=2, space="PSUM") as ps1pool, \
             tc.tile_pool(name="ps2", bufs=1, space="PSUM") as ps2pool, \
             tc.tile_pool(name="pst", bufs=2, space="PSUM") as pstpool:

            # ---- resident weights: [128, ktile*H] with ktile-major free layout
            w1t_sb = cpool.tile([128, KH * H], DT)
            w2t_sb = cpool.tile([128, KH * H], DT)
            wi1t_sb = cpool.tile([128, KI * H], DT)
            wi2t_sb = cpool.tile([128, KI * H], DT)
            id_sb = cpool.tile([64, 64], DT)
            nc.sync.dma_start(id_sb[:], IDN[:])
            for k in range(KH):
                nc.sync.dma_start(w1t_sb[:, k*H:(k+1)*H], W1T[k*128:(k+1)*128, :])
                nc.sync.dma_start(w2t_sb[:, k*H:(k+1)*H], W2T[k*128:(k+1)*128, :])
            for k in range(KI):
                nc.sync.dma_start(wi1t_sb[:, k*H:(k+1)*H], Wi1T[k*128:(k+1)*128, :])
                nc.sync.dma_start(wi2t_sb[:, k*H:(k+1)*H], Wi2T[k*128:(k+1)*128, :])
            if with_bias:
                bia1_sb = cpool.tile([1, H], DT)
                bia2_sb = cpool.tile([1, H], DT)
                one_sb = cpool.tile([1, BC], DT)
                nc.sync.dma_start(bia1_sb[:], BIA[0:1, :])
                nc.sync.dma_start(bia2_sb[:], BIA[1:2, :])
                nc.sync.dma_start(one_sb[:], ONE[:])

            # ---- XT prefetch
            xts = {}

            def fetch_xt(l):
                if l >= L_steps:
                    return
                t = xpool.tile([128, KI * BC], DT, tag="xt")
                nc.sync.dma_start(t[:], XT[l])
                xts[l] = t

            class Group:
                """One step's PSUM accumulation.  PE column strip g streams
                output columns [g*512,(g+1)*512) of every entry into its own
                PSUM bank: ps[g*64:(g+1)*64, g*512:(g+1)*512].  The two strip
                chains are emitted interleaved so they run concurrently.

                Entry order: [bias?] + KI x-tiles + ZORD z-tiles.  bias/X are
                state-independent and can be emitted early; zT is set before
                the z entries are emitted."""

                def __init__(self, ps, xt_t, wi_sb, w_sb, bias_sb, nz=KH):
                    self.ps, self.xt, self.wi, self.w = ps, xt_t, wi_sb, w_sb
                    self.bias = bias_sb
                    self.zT = None
                    self.nz = nz
                    self.done = 0

                @property
                def nb(self):
                    return 1 if self.bias is not None else 0

                @property
                def n_open(self):
                    return self.nb + KI   # state-independent prefix

                def entry(self, i):
                    if i < self.nb:
                        return one_sb[0:1, :], self.bias, 0
                    i -= self.nb
                    if i < KI:
                        return self.xt[:, i*BC:(i+1)*BC], self.wi, i
                    k = ZORD[i - KI]
                    return self.zT[:, k*BC:(k+1)*BC], self.w, k

                def emit(self, hi=None):
                    n = self.nb + KI + self.nz
                    hi = n if hi is None else min(hi, n)
                    for i in range(self.done, hi):
                        stat, mov, k = self.entry(i)
                        for g in range(2):
                            nc.tensor.matmul(
                                self.ps[g*BC:(g+1)*BC, g*512:(g+1)*512],
                                stat, mov[:, k*H + g*512: k*H + g*512 + 512],
                                start=(i == 0), stop=(i == n - 1),
                                tile_position=(0, g*BC))
                    self.done = max(self.done, hi)

            def open_z1(l, nz=KH):
                ps = ps1pool.tile([128, H], F32, tag="ps1")
                return Group(ps, xts[l], wi1t_sb, w1t_sb,
                             bia1_sb if with_bias else None, nz)

            def open_z2(l, zT, nz=KH):
                ps = ps2pool.tile([128, H], F32, tag="ps2")
                g = Group(ps, xts[l], wi2t_sb, w2t_sb,
                          bia2_sb if with_bias else None, nz)
                g.zT = zT
                return g

            def tanh_step(ps, halves, nchunk=2):
                """halves[h][:, c] = tanh(ps[h*64:(h+1)*64, h*512 + c]):
                half h holds z[:, h*512:(h+1)*512] on partitions 0-63.
                Chunk order (h0,c0),(h1,c0),(h0,c1),(h1,c1) so the transposes
                for KORD k-tiles become ready progressively."""
                cw = 512 // nchunk
                for c in range(nchunk):
                    for h in range(2):
                        nc.scalar.activation(
                            halves[h][:, c*cw:(c+1)*cw],
                            ps[h*BC:(h+1)*BC, h*512 + c*cw: h*512 + (c+1)*cw],
                            Tanh)

            def transposes(halves, pst):
                # halves[h][b, c] = z[b, h*512 + c]
                for kk in KORD:
                    h, j = kk // 4, kk % 4
                    nc.tensor.transpose(pst[:, kk*BC:(kk+1)*BC],
                                        halves[h][:, j*128:(j+1)*128],
                                        id_sb[:])

            def z2_post(g2):
                """tanh + transposes + copy -> new pending z2T tile."""
                z2n = [apool.tile([BC, 512], DT, tag="z2na", name="z2na"),
                       apool.tile([BC, 512], DT, tag="z2nb", name="z2nb")]
                tanh_step(g2.ps, z2n, nchunk=1)
                pst2 = pstpool.tile([128, KH * BC], DT, tag="pst")
                transposes(z2n, pst2)
                z2T_new = spool.tile([128, KH * BC], DT, tag="z2T")
                nc.vector.tensor_copy(z2T_new[:], pst2[:])
                return z2T_new

            def add_z12(pst1, z2T):
                z12T = spool.tile([128, KH * BC], DT, tag="z12T")
                for p in range(0, KH, 2):
                    a = KORD[p]
                    nc.vector.tensor_add(z12T[:, a*BC:(a+2)*BC],
                                         pst1[:, a*BC:(a+2)*BC],
                                         z2T[:, a*BC:(a+2)*BC])
                return z12T

            def body():
                nonlocal xts
                xts = {}
                for l in range(min(3, L_steps)):
                    fetch_xt(l)
                # step 0: no recurrent state -> X(+bias)-only groups
                g1 = open_z1(0, nz=0)
                g1.emit()
                g2 = open_z2(0, None, nz=0)
                g2.emit()
                z2T_pending = z2_post(g2)
                g2 = None
                z2T = None

                for l in range(L_steps):
                    even = (l % 2 == 0)
                    last = (l == L_steps - 1)
                    fetch_xt(l + 3)

                    # close this step's z1 accumulation
                    g1.emit()

                    # z2 state after step l (updated on even steps)
                    if even:
                        z2T = z2T_pending

                    # finish the z2 matmul group for step l+1 (PE fill)
                    if (not last) and (l % 2 == 1) and g2 is not None:
                        g2.emit()

                    # tanh of this step's z1
                    if last:
                        fin = [fpool.tile([BC, 512], F32, tag="fina", name="fina"),
                               fpool.tile([BC, 512], F32, tag="finb", name="finb")]
                        tanh_step(g1.ps, fin, nchunk=1)
                        nc.sync.dma_start(OUT[:, 0:512], fin[0][:])
                        nc.sync.dma_start(OUT[:, 512:1024], fin[1][:])
                        break
                    z1n = [apool.tile([BC, 512], DT, tag="z1na", name="z1na"),
                           apool.tile([BC, 512], DT, tag="z1nb", name="z1nb")]
                    tanh_step(g1.ps, z1n)

                    # open next step's z1 group; emit state-independent prefix
                    g1n = open_z1(l + 1)
                    g1n.emit(g1n.n_open)

                    # open the z2 group for step l+2 at the even-step tail --
                    # ahead of the tanh-gated transposes in the PE FIFO, so its
                    # ready entries fill the tanh/transpose/add wait
                    if even and l + 2 < L_steps:
                        g2 = open_z2(l + 2, z2T)
                        g2.emit(g2.nb + N_Z2A)

                    # transpose z1n -> z1nT k-tiles (PSUM)
                    pst1 = pstpool.tile([128, KH * BC], DT, tag="pst")
                    transposes(z1n, pst1)

                    # z2 epilogue for step l+1 (tanh_z2 queues behind tanh_z1 on
                    # ACT; its transposes fill the PE while DVE adds below)
                    if (not last) and (l % 2 == 1) and g2 is not None:
                        z2T_pending = z2_post(g2)
                        g2 = None

                    # z12T = z1nT + z2T(after this step)
                    z12T = add_z12(pst1, z2T)
                    g1n.zT = z12T
                    g1 = g1n

                    if l >= 1:
                        xts.pop(l - 1, None)

            if reps > 1:
                with tc.For_i(0, reps, 1):
                    body()
            else:
                body()
    nc.compile()
    return nc


def _get_nc(L_steps, with_bias, reps=1):
    key = (L_steps, with_bias, reps)
    if key not in _CACHE:
        _CACHE[key] = _build(L_steps, with_bias, reps)
    return _CACHE[key]


def _prep_in_maps(X, W_in1, b_in1, W_rec1, W_in2, b_in2, W_rec2, L_steps):
    dt = np.float16
    with_bias = bool(np.any(b_in1) or np.any(b_in2))
    w1t = np.ascontiguousarray(W_rec1.T.astype(dt))
    w2t = np.ascontiguousarray(W_rec2.T.astype(dt))
    wi1t = np.ascontiguousarray(W_in1.T.astype(dt))
    wi2t = np.ascontiguousarray(W_in2.T.astype(dt))
    idn = np.eye(64, dtype=dt)
    in_maps = []
    for c in range(NC):
        xt = np.ascontiguousarray(
            X[c*BC:(c+1)*BC, :L_steps, :].transpose(1, 2, 0)
            .reshape(L_steps, KI, 128, BC).transpose(0, 2, 1, 3)
            .reshape(L_steps, 128, KI * BC).astype(dt))
        m = {"XT": xt, "W1T": w1t, "W2T": w2t, "Wi1T": wi1t, "Wi2T": wi2t,
             "IDN": idn}
        if with_bias:
            m["BIA"] = np.ascontiguousarray(
                np.stack([b_in1[:, 0], b_in2[:, 0]]).astype(dt))
            m["ONE"] = np.ones((1, BC), dt)
        in_maps.append(m)
    return in_maps, with_bias


def run_device(X, W_in1, b_in1, W_rec1, W_in2, b_in2, W_rec2, L_steps=L):
    """Run the recurrence on 8 cores; returns z1_final (B, H) float32."""
    from concourse.bass_utils import run_bass_kernel_spmd
    in_maps, with_bias = _prep_in_maps(X, W_in1, b_in1, W_rec1, W_in2, b_in2,
                                       W_rec2, L_steps)
    nc = _get_nc(L_steps, with_bias, 1)
    res = run_bass_kernel_spmd(nc, in_maps, list(range(NC)))
    return np.concatenate([res.results[c]["OUT"] for c in range(NC)], axis=0)


def kernel(X, W_in1, b_in1, W_rec1, W_in2, b_in2, W_rec2, W_out, b_out):
    X = np.asarray(X); W_out = np.asarray(W_out); b_out = np.asarray(b_out)
    assert X.shape == (B, L, I), f"unexpected X shape {X.shape}"
    z1 = run_device(X, np.asarray(W_in1), np.asarray(b_in1),
                    np.asarray(W_rec1), np.asarray(W_in2), np.asarray(b_in2),
                    np.asarray(W_rec2))
    out = np.tanh(z1.astype(np.float64) @ W_out.astype(np.float64).T
                  + b_out.astype(np.float64)[:, 0])
    return out.reshape(B, 1).astype(np.float32)


# revision 11
# speedup vs baseline: 1.0846x; 1.0585x over previous
"""Trainium2 Bass kernel for nn_AlarmworkRNN: 2-track tanh RNN (v2).

Math (per reference):
  for l in 0..L-1:
      z1n = tanh(X[:,l] @ W_in1.T + b1 + (z1 + z2) @ W_rec1.T)
      z2n = tanh(X[:,l] @ W_in2.T + b2 + z2 @ W_rec2.T)  if l even else z2
      z1, z2 = z1n, z2n
  out = tanh(z1 @ W_out.T + b_out)       (computed on host, O=1)

Strategy (v2, fp16, "output-split" column tiling):
  Data-parallel over batch (8 cores x 64 rows).  State is held transposed
  (z12T, z2T: [H=1024 -> 8 k-tiles of 128, B=64]) as the matmul stationary;
  host-pretransposed weights are the moving operand, resident in SBUF.  The
  input projection X[l] @ W_in.T joins the same PSUM accumulation as 2 extra
  k-tiles (stationary = host-pretransposed X[l].T).

  Unlike v1 (which split the k-entries across the two PE column groups so the
  chains had to run serially per step and the two PSUM halves then needed an
  ACT copy + DVE add), v2 splits the OUTPUT columns: PE column strip g
  computes output columns [g*512,(g+1)*512) for ALL k-tiles.  Each strip's
  accumulation chain lives in its own PSUM bank (one pending group per bank
  is a hard HW/sim rule), and the two chains are emitted interleaved so they
  stream CONCURRENTLY through the two halves of the PE array -- per step the
  PE streams 10 x 512 columns instead of 20 x 512.

  The step's pre-activation lands "diagonally": bank0 partitions 0-63 hold
  cols 0-511, bank1 partitions 64-127 hold cols 512-1023.  ACT tanh maps
  both into one "stacked" SBUF tile [128, 512] (chunked so transposes/adds
  pipeline); PE transposes turn the stacked halves into z1nT k-tiles and a
  DVE add (z1nT + z2T) forms the next stationary.  The z-entry order
  (0,1,4,5,2,3,6,7) matches tanh-chunk completion so the next step's matmuls
  become ready progressively.

  The z2 track updates only on even steps; its matmul group is emitted
  split around the z1 chain as PE fill for the tanh/transpose/add latency.
"""
import numpy as np

B, L, I, H = 512, 512, 256, 1024
NC = 8
BC = B // NC          # 64 batch rows per core
KH = H // 128         # 8 hidden k-tiles
KI = I // 128         # 2 input k-tiles

N_Z2A = 5             # z2-group entries emitted at the opening (even) step
ZORD = (0, 1, 4, 5, 2, 3, 6, 7)   # z-entry order (tanh-chunk completion order)
KORD = (0, 1, 4, 5, 2, 3, 6, 7)   # transpose/add k-tile order

_CACHE = {}


def _build(L_steps, with_bias, reps=1):
    import concourse.bacc as bacc
    import concourse.tile as tile
    import concourse.mybir as mybir

    F32 = mybir.dt.float32
    DT = mybir.dt.float16
    Tanh = mybir.ActivationFunctionType.Tanh

    nc = bacc.Bacc("TRN2", target_bir_lowering=False)
    XT = nc.declare_dram_parameter("XT", [L_steps, 128, KI * BC], DT, isOutput=False)
    W1T = nc.declare_dram_parameter("W1T", [H, H], DT, isOutput=False)
    W2T = nc.declare_dram_parameter("W2T", [H, H], DT, isOutput=False)
    Wi1T = nc.declare_dram_parameter("Wi1T", [I, H], DT, isOutput=False)
    Wi2T = nc.declare_dram_parameter("Wi2T", [I, H], DT, isOutput=False)
    IDN = nc.declare_dram_parameter("IDN", [64, 64], DT, isOutput=False)
    if with_bias:
        BIA = nc.declare_dram_parameter("BIA", [2, H], DT, isOutput=False)
        ONE = nc.declare_dram_parameter("ONE", [1, BC], DT, isOutput=False)
    OUT = nc.declare_dram_parameter("OUT", [BC, H], F32, isOutput=True)

    with tile.TileContext(nc) as tc:
        with tc.tile_pool(name="const", bufs=1) as cpool, \
             tc.tile_pool(name="xt", bufs=6) as xpool, \
             tc.tile_pool(name="st", bufs=3) as spool, \
             tc.tile_pool(name="actt", bufs=3) as apool, \
             tc.tile_pool(name="fin", bufs=1) as fpool, \
             tc.tile_pool(name="ps1", bufs=2, space="PSUM") as ps1pool, \
             tc.tile_pool(name="ps2", bufs=1, space="PSUM") as ps2pool, \
             tc.tile_pool(name="pst", bufs=2, space="PSUM") as pstpool:

            # ---- resident weights: [128, ktile*H] with ktile-major free layout
            w1t_sb = cpool.tile([128, KH * H], DT)
            w2t_sb = cpool.tile([128, KH * H], DT)
            wi1t_sb = cpool.tile([128, KI * H], DT)
            wi2t_sb = cpool.tile([128, KI * H], DT)
            id_sb = cpool.tile([64, 64], DT)
            nc.sync.dma_start(id_sb[:], IDN[:])
            for k in range(KH):
                nc.sync.dma_start(w1t_sb[:, k*H:(k+1)*H], W1T[k*128:(k+1)*128, :])
                nc.sync.dma_start(w2t_sb[:, k*H:(k+1)*H], W2T[k*128:(k+1)*128, :])
            for k in range(KI):
                nc.sync.dma_start(wi1t_sb[:, k*H:(k+1)*H], Wi1T[k*128:(k+1)*128, :])
                nc.sync.dma_start(wi2t_sb[:, k*H:(k+1)*H], Wi2T[k*128:(k+1)*128, :])
            if with_bias:
                bia1_sb = cpool.tile([1, H], DT)
                bia2_sb = cpool.tile([1, H], DT)
                one_sb = cpool.tile([1, BC], DT)
                nc.sync.dma_start(bia1_sb[:], BIA[0:1, :])
                nc.sync.dma_start(bia2_sb[:], BIA[1:2, :])
                nc.sync.dma_start(one_sb[:], ONE[:])

            # ---- XT prefetch
            xts = {}

            def fetch_xt(l):
                if l >= L_steps:
                    return
                t = xpool.tile([128, KI * BC], DT, tag="xt")
                nc.sync.dma_start(t[:], XT[l])
                xts[l] = t

            class Group:
                """One step's PSUM accumulation.  PE column strip g streams
                output columns [g*512,(g+1)*512) of every entry into its own
                PSUM bank: ps[g*64:(g+1)*64, g*512:(g+1)*512].  The two strip
                chains are emitted interleaved so they run concurrently.

                Entry order: [bias?] + KI x-tiles + ZORD z-tiles.  bias/X are
                state-independent and can be emitted early; zT is set before
                the z entries are emitted."""

                def __init__(self, ps, xt_t, wi_sb, w_sb, bias_sb, nz=KH):
                    self.ps, self.xt, self.wi, self.w = ps, xt_t, wi_sb, w_sb
                    self.bias = bias_sb
                    self.zT = None
                    self.nz = nz
                    self.done = 0

                @property
                def nb(self):
                    return 1 if self.bias is not None else 0

                @property
                def n_open(self):
                    return self.nb + KI   # state-independent prefix

                def entry(self, i):
                    if i < self.nb:
                        return one_sb[0:1, :], self.bias, 0
                    i -= self.nb
                    if i < KI:
                        return self.xt[:, i*BC:(i+1)*BC], self.wi, i
                    k = ZORD[i - KI]
                    return self.zT[:, k*BC:(k+1)*BC], self.w, k

                def emit(self, hi=None):
                    n = self.nb + KI + self.nz
                    hi = n if hi is None else min(hi, n)
                    for i in range(self.done, hi):
                        stat, mov, k = self.entry(i)
                        for g in range(2):
                            nc.tensor.matmul(
                                self.ps[g*BC:(g+1)*BC, g*512:(g+1)*512],
                                stat, mov[:, k*H + g*512: k*H + g*512 + 512],
                                start=(i == 0), stop=(i == n - 1),
                                tile_position=(0, g*BC))
                    self.done = max(self.done, hi)

            def open_z1(l, nz=KH):
                ps = ps1pool.tile([128, H], F32, tag="ps1")
                return Group(ps, xts[l], wi1t_sb, w1t_sb,
                             bia1_sb if with_bias else None, nz)

            def open_z2(l, zT, nz=KH):
                ps = ps2pool.tile([128, H], F32, tag="ps2")
                g = Group(ps, xts[l], wi2t_sb, w2t_sb,
                          bia2_sb if with_bias else None, nz)
                g.zT = zT
                return g

            def tanh_step(ps, halves, nchunk=2):
                """halves[h][:, c] = tanh(ps[h*64:(h+1)*64, h*512 + c]):
                half h holds z[:, h*512:(h+1)*512] on partitions 0-63.
                Chunk order (h0,c0),(h1,c0),(h0,c1),(h1,c1) so the transposes
                for KORD k-tiles become ready progressively."""
                cw = 512 // nchunk
                for c in range(nchunk):
                    for h in range(2):
                        nc.scalar.activation(
                            halves[h][:, c*cw:(c+1)*cw],
                            ps[h*BC:(h+1)*BC, h*512 + c*cw: h*512 + (c+1)*cw],
                            Tanh)

            def transposes(halves, pst):
                # halves[h][b, c] = z[b, h*512 + c]
                for kk in KORD:
                    h, j = kk // 4, kk % 4
                    nc.tensor.transpose(pst[:, kk*BC:(kk+1)*BC],
                                        halves[h][:, j*128:(j+1)*128],
                                        id_sb[:])

            def z2_tanh(g2):
                """z2 tanh only (queues behind z1's chunks on ACT)."""
                z2n = [apool.tile([BC, 512], DT, tag="z2na", name="z2na"),
                       apool.tile([BC, 512], DT, tag="z2nb", name="z2nb")]
                tanh_step(g2.ps, z2n, nchunk=1)
                return z2n

            def z2_finish(z2n):
                """transposes + copy -> new z2T tile, emitted one step AFTER
                the tanh: in the PE FIFO these transposes are ready work
                (fill right behind the close) instead of blocking the queue
                until z2's late tanh completes."""
                pst2 = pstpool.tile([128, KH * BC], DT, tag="pst")
                transposes(z2n, pst2)
                z2T_new = spool.tile([128, KH * BC], DT, tag="z2T")
                nc.vector.tensor_copy(z2T_new[:], pst2[:])
                return z2T_new

            def add_z12(pst1, z2T):
                z12T = spool.tile([128, KH * BC], DT, tag="z12T")
                for p in range(0, KH, 2):
                    a = KORD[p]
                    nc.vector.tensor_add(z12T[:, a*BC:(a+2)*BC],
                                         pst1[:, a*BC:(a+2)*BC],
                                         z2T[:, a*BC:(a+2)*BC])
                return z12T

            def body():
                nonlocal xts
                xts = {}
                for l in range(min(3, L_steps)):
                    fetch_xt(l)
                # step 0: no recurrent state -> X(+bias)-only groups
                g1 = open_z1(0, nz=0)
                g1.emit()
                g2 = open_z2(0, None, nz=0)
                g2.emit()
                z2n_pending = z2_tanh(g2)
                g2 = None
                z2T = None

                for l in range(L_steps):
                    even = (l % 2 == 0)
                    last = (l == L_steps - 1)
                    fetch_xt(l + 3)

                    # close this step's z1 accumulation
                    g1.emit()

                    # z2 state after step l (updated on even steps):
                    # transposes of the (long-done) z2 tanh + DVE copy
                    if even:
                        z2T = z2_finish(z2n_pending)

                    # finish the z2 matmul group for step l+1 (PE fill)
                    if (not last) and (l % 2 == 1) and g2 is not None:
                        g2.emit()

                    # tanh of this step's z1
                    if last:
                        fin = [fpool.tile([BC, 512], F32, tag="fina", name="fina"),
                               fpool.tile([BC, 512], F32, tag="finb", name="finb")]
                        tanh_step(g1.ps, fin, nchunk=1)
                        nc.sync.dma_start(OUT[:, 0:512], fin[0][:])
                        nc.sync.dma_start(OUT[:, 512:1024], fin[1][:])
                        break
                    z1n = [apool.tile([BC, 512], DT, tag="z1na", name="z1na"),
                           apool.tile([BC, 512], DT, tag="z1nb", name="z1nb")]
                    tanh_step(g1.ps, z1n)

                    # open next step's z1 group; emit state-independent prefix
                    g1n = open_z1(l + 1)
                    g1n.emit(g1n.n_open)

                    # open the z2 group for step l+2 at the even-step tail --
                    # ahead of the tanh-gated transposes in the PE FIFO, so its
                    # ready entries fill the tanh/transpose/add wait
                    if even and l + 2 < L_steps:
                        g2 = open_z2(l + 2, z2T)
                        g2.emit(g2.nb + N_Z2A)

                    # transpose z1n -> z1nT k-tiles (PSUM)
                    pst1 = pstpool.tile([128, KH * BC], DT, tag="pst")
                    transposes(z1n, pst1)

                    # z2 epilogue (tanh only; transposes deferred to the next
                    # even step's fill)
                    if (not last) and (l % 2 == 1) and g2 is not None:
                        z2n_pending = z2_tanh(g2)
                        g2 = None

                    # z12T = z1nT + z2T(after this step)
                    z12T = add_z12(pst1, z2T)
                    g1n.zT = z12T
                    g1 = g1n

                    if l >= 1:
                        xts.pop(l - 1, None)

            if reps > 1:
                with tc.For_i(0, reps, 1):
                    body()
            else:
                body()
    nc.compile()
    return nc


def _get_nc(L_steps, with_bias, reps=1):
    key = (L_steps, with_bias, reps)
    if key not in _CACHE:
        _CACHE[key] = _build(L_steps, with_bias, reps)
    return _CACHE[key]


def _prep_in_maps(X, W_in1, b_in1, W_rec1, W_in2, b_in2, W_rec2, L_steps):
    dt = np.float16
    with_bias = bool(np.any(b_in1) or np.any(b_in2))
    w1t = np.ascontiguousarray(W_rec1.T.astype(dt))
    w2t = np.ascontiguousarray(W_rec2.T.astype(dt))
    wi1t = np.ascontiguousarray(W_in1.T.astype(dt))
    wi2t = np.ascontiguousarray(W_in2.T.astype(dt))
    idn = np.eye(64, dtype=dt)
    in_maps = []
    for c in range(NC):
        xt = np.ascontiguousarray(
            X[c*BC:(c+1)*BC, :L_steps, :].transpose(1, 2, 0)
            .reshape(L_steps, KI, 128, BC).transpose(0, 2, 1, 3)
            .reshape(L_steps, 128, KI * BC).astype(dt))
        m = {"XT": xt, "W1T": w1t, "W2T": w2t, "Wi1T": wi1t, "Wi2T": wi2t,
             "IDN": idn}
        if with_bias:
            m["BIA"] = np.ascontiguousarray(
                np.stack([b_in1[:, 0], b_in2[:, 0]]).astype(dt))
            m["ONE"] = np.ones((1, BC), dt)
        in_maps.append(m)
    return in_maps, with_bias


def run_device(X, W_in1, b_in1, W_rec1, W_in2, b_in2, W_rec2, L_steps=L):
    """Run the recurrence on 8 cores; returns z1_final (B, H) float32."""
    from concourse.bass_utils import run_bass_kernel_spmd
    in_maps, with_bias = _prep_in_maps(X, W_in1, b_in1, W_rec1, W_in2, b_in2,
                                       W_rec2, L_steps)
    nc = _get_nc(L_steps, with_bias, 1)
    res = run_bass_kernel_spmd(nc, in_maps, list(range(NC)))
    return np.concatenate([res.results[c]["OUT"] for c in range(NC)], axis=0)


def kernel(X, W_in1, b_in1, W_rec1, W_in2, b_in2, W_rec2, W_out, b_out):
    X = np.asarray(X); W_out = np.asarray(W_out); b_out = np.asarray(b_out)
    assert X.shape == (B, L, I), f"unexpected X shape {X.shape}"
    z1 = run_device(X, np.asarray(W_in1), np.asarray(b_in1),
                    np.asarray(W_rec1), np.asarray(W_in2), np.asarray(b_in2),
                    np.asarray(W_rec2))
    out = np.tanh(z1.astype(np.float64) @ W_out.astype(np.float64).T
                  + b_out.astype(np.float64)[:, 0])
    return out.reshape(B, 1).astype(np.float32)
